# revision 58
# baseline (speedup 1.0000x reference)
# Distributed Trainium2 kernel for the QKV-MLP + causal multi-head attention layer.
#
# Problem (all shapes fixed):
#   x [2, 2048, 1024];  q/k/v = MLP(x) with w1 [1024, 4096] (silu) and w2 [4096, 1024]
#   16 heads x 64 dims, causal softmax attention, output [2, 2048, 1024].
#
# Sharding (8 NeuronCores, single SPMD program; per-core behavior differs only
# through per-core input DATA, never through the instruction stream):
#   - Token/data parallel MLPs: each core computes q/k/v for 512 tokens of one
#     batch (cores 0-3 -> batch 0, 4-7 -> batch 1) with full (replicated) weights.
#   - Core quarter cc owns query tiles [cc, 7-cc, 8+cc, 15-cc] (128 tokens each),
#     which balances causal attention cost (sum of key-tiles = 34 for every cc).
#   - k^T (D-major) and v (token-major, with a ones-column per head for the
#     softmax denominator) are AllGathered in 4 chunks each within the 4-core
#     batch group, as soon as each chunk lands in DRAM.
#
# Precision:
#   - q,k MLPs run in fp8 e4m3 with DoubleRow matmuls (2 fp8 MACs/cell/cycle):
#     weights are scaled by 32 on the host (de-scaled via activation scale),
#     activations quantized to e4m3 (32*silu).  Score errors this introduces
#     mostly wash out in the softmax.
#   - v MLP stays bf16: the output inherits v errors directly.
#   - Attention is bf16 with fp32 PSUM accumulation.
#
# Attention engine schedule (software-pipelined by one head-pair):
#   - S^T = k^T q per (head-pair, key-tile-pair) into 2-bank PSUM, exp on
#     ScalarE (ex tiles stored at causal width), diagonal-slot mask on VectorE.
#   - PV is FLIPPED: stationary = v tile [key,65] (with ones column), moving =
#     exp tiles [key, q] -> out^T [65, q] accumulated over 16 key tiles in one
#     PSUM bank.  PV(p) only reads ex tiles masked during iteration p-1, so it
#     never waits on ScalarE; S^T(p+1) units are woven between PV(p) chunks.
#   - softmax division: 1/den via Ln+Exp on ScalarE refined by one
#     Newton-Raphson step on GpSimd, broadcast across partitions with a tiny
#     fp16 matmul into the free partitions 64..127 of the same PSUM bank, one
#     VectorE multiply, store.  No engine sees more than ~1us per head.
#   - Output is written D-major [D, TOK] and transposed on the host.

import numpy as np
import ml_dtypes

B, S, D, HID, H, DH = 2, 2048, 1024, 4096, 16, 64
P = 128
NCORES = 8
TOK = 512           # tokens per core
NT = S // P         # 16 query/key tiles per batch
VROW = H * (DH + 1)  # 1040: v row with a ones column per head
WS = 32.0           # fp8 weight pre-scale (power of two; exact in fp8)

BF16 = ml_dtypes.bfloat16
F8 = ml_dtypes.float8_e4m3  # TRN fp8_exp4-compatible (max normal 240)

_CACHE = {}


def _tiles_for(cc):
    """Global query-tile indices (slot order) owned by quarter cc."""
    return [cc, 7 - cc, 8 + cc, 15 - cc]


def _owner(j):
    """Global tile j -> (owner quarter, slot index)."""
    blk = j // 4
    own = [j, 7 - j, j - 8, 15 - j][blk]
    return own, blk


def _build_program():
    import concourse.bass as bass
    import concourse.mybir as mybir
    import concourse.tile as tile
    from concourse import bacc

    dt = mybir.dt
    AF = mybir.ActivationFunctionType
    DR = mybir.MatmulPerfMode.DoubleRow
    ALU = mybir.AluOpType

    # Bacc (not raw Bass): its compile() splits multi-sem waits into event
    # semaphores, which TRN2 engine instructions require (max 1 wait each).
    nc = bacc.Bacc("TRN2", num_devices=NCORES)

    # ---- I/O ----
    xT16_d = nc.dram_tensor("xT16", [P, 8, TOK], dt.bfloat16, kind="ExternalInput")
    xT8_d = nc.dram_tensor("xT8", [P, 8, TOK], dt.float8e4, kind="ExternalInput")
    w1v_d = nc.dram_tensor("w1v", [HID // P, P, 8, P], dt.bfloat16, kind="ExternalInput")
    # fp8 w1: HID-tile PAIRS with the pair inside the partition line (2KB DMAs)
    w1kq_d = {
        m: nc.dram_tensor(f"w1{m}", [HID // (2 * P), P, 2, 8, P], dt.float8e4,
                          kind="ExternalInput")
        for m in "kq"
    }
    w2kq_d = {
        m: nc.dram_tensor(f"w2{m}", [D // P, P, HID // P, P], dt.float8e4,
                          kind="ExternalInput")
        for m in "kq"
    }
    w2v_d = nc.dram_tensor("w2v", [HID // P, P, D], dt.bfloat16, kind="ExternalInput")
    b1_d = nc.dram_tensor("b1", [P, 96], dt.float32, kind="ExternalInput")
    b1s_d = nc.dram_tensor("b1s", [P, 96], dt.float32, kind="ExternalInput")
    b2kq_d = nc.dram_tensor("b2kq", [P, 16], dt.float32, kind="ExternalInput")
    b2v_d = nc.dram_tensor("b2v", [1, D], dt.bfloat16, kind="ExternalInput")
    masks_d = nc.dram_tensor("masks", [P, NT // 2, 2, P], dt.bfloat16, kind="ExternalInput")
    o_d = nc.dram_tensor("o", [D, TOK], dt.float32, kind="ExternalOutput")

    with tile.TileContext(nc) as tc:
        with (
            tc.tile_pool(name="constp", bufs=1) as constp,
            tc.tile_pool(name="w1p", bufs=8) as w1p,
            tc.tile_pool(name="w2p", bufs=8) as w2p,
            tc.tile_pool(name="hp8", bufs=25) as hp8,
            tc.tile_pool(name="qtp", bufs=2) as qtp,
            tc.tile_pool(name="kstp", bufs=3) as kstp,
            tc.tile_pool(name="psp", bufs=2, space="PSUM") as psp,
            tc.tile_pool(name="dramp", bufs=1, space="DRAM") as dramp,
        ):
            # DRAM bounce buffers for the collectives
            k_dram = dramp.tile([D, TOK], dt.bfloat16, tag="k_dram")
            v_dram = dramp.tile([TOK, VROW], dt.bfloat16, tag="v_dram")
            kag_dram = dramp.tile([4 * D, TOK], dt.bfloat16, tag="kag_dram")
            vag_dram = dramp.tile([4 * TOK, VROW], dt.bfloat16, tag="vag_dram")

            groups = [[0, 1, 2, 3], [4, 5, 6, 7]]

            def mlp1_fp8_unit(w1d, b1col0, mp, hts):
                """One HID-tile pair of the fp8 first linear: appends an
                e4m3(32*silu) tile [P, 2, TOK] to hts (pair dim for DoubleRow)."""
                w1t = w1p.tile([P, 2, 8, P], dt.float8e4, tag="w1t8")
                nc.sync.dma_start(w1t, w1d[mp, :, :, :, :])
                hpair = hp8.tile([P, 2, TOK], dt.float8e4, tag="h8")
                for mi in range(2):
                    m = 2 * mp + mi
                    ps = psp.tile([P, TOK], dt.float32, tag="ps1", bufs=4)
                    for i in range(4):
                        nc.tensor.matmul(
                            ps, lhsT=w1t[:, mi, 2 * i : 2 * i + 2, :],
                            rhs=xt8[:, 2 * i : 2 * i + 2, :],
                            start=(i == 0), stop=(i == 3), perf_mode=DR,
                        )
                    # ps = 32*(x@w1);  sigmoid(ps/32 + b1),  h = (ps+32*b1)*sg
                    bcol = b1_sb[:, b1col0 + m : b1col0 + m + 1]
                    bscol = b1s_sb[:, b1col0 + m : b1col0 + m + 1]
                    sg = kstp.tile([P, TOK], dt.float32, tag="sg")
                    nc.scalar.activation(sg, ps, AF.Sigmoid, bias=bcol, scale=1.0 / WS)
                    nc.vector.scalar_tensor_tensor(
                        hpair[:, mi, :], ps, bscol, sg,
                        op0=ALU.add, op1=ALU.mult,
                    )
                hts.append(hpair)

            def w2_load(w2d, md, pre):
                """Prefetch both 2KB/partition halves of one w2 output tile."""
                for half in range(2):
                    w2t = w2p.tile([P, HID // (2 * P), P], dt.float8e4, tag="w2t8")
                    nc.sync.dma_start(w2t, w2d[md, :, 16 * half : 16 * half + 16, :])
                    pre.append(w2t)

            def mlp2_fp8_md(b2col0, hts, md, dest, pre):
                """One 128-row output tile of the second linear (fp8 DoubleRow):
                ps = (32*silu_h) @ (32*w2); out = Identity(ps/1024 + b2)."""
                ps = psp.tile([P, TOK], dt.float32, tag="ps1", bufs=4)
                for half in range(2):
                    w2t = pre.pop(0)
                    for i in range(8):
                        kp = 8 * half + i
                        nc.tensor.matmul(
                            ps, lhsT=w2t[:, 2 * i : 2 * i + 2, :], rhs=hts[kp],
                            start=(kp == 0), stop=(kp == 15), perf_mode=DR,
                        )
                bcol = b2kq_sb[:, b2col0 + md : b2col0 + md + 1]
                if dest == "k":
                    # bias+descale on DVE (ScalarE is the busier engine here)
                    kst = kstp.tile([P, TOK], dt.bfloat16, tag="kst")
                    nc.vector.tensor_scalar(
                        kst, ps, 1.0 / (WS * WS), bcol,
                        op0=ALU.mult, op1=ALU.add,
                    )
                    # store on the scalar queue: on gpsimd it would queue
                    # behind an AllGather (inter-core wait) and starve the
                    # kst pool -> DVE -> PSUM -> PE chain for ~10us
                    nc.scalar.dma_start(k_dram[P * md : P * (md + 1), :], kst)
                    if md % 2 == 1:
                        # AllGather this 256-row chunk of k^T right away
                        nc.gpsimd.collective_compute(
                            "AllGather", ALU.bypass,
                            replica_groups=groups,
                            ins=[k_dram[P * (md - 1) : P * (md + 1), :].opt()],
                            outs=[kag_dram[4 * P * (md - 1) : 4 * P * (md + 1), :].opt()],
                        )
                    return None
                qt = qtp.tile([P, TOK], dt.bfloat16, tag="qt")
                nc.vector.tensor_scalar(
                    qt, ps, 1.0 / (WS * WS), bcol, op0=ALU.mult, op1=ALU.add,
                )
                return qt

            # ================= v MLP (bf16), scoped pools =================
            with (
                tc.tile_pool(name="vconst", bufs=1) as vconst,
                tc.tile_pool(name="w1vp", bufs=2) as w1vp,
                tc.tile_pool(name="w2vp", bufs=6) as w2vp,
                tc.tile_pool(name="hp", bufs=32) as hp,
                tc.tile_pool(name="vstp", bufs=2) as vstp,
            ):
                xt = vconst.tile([P, 8, TOK], dt.bfloat16, tag="xt")
                nc.sync.dma_start(xt[:, 0:2, :], xT16_d[:, 0:2, :])
                b1_sb = constp.tile([P, 96], dt.float32, tag="b1")
                nc.sync.dma_start(b1_sb, b1_d[:, :])

                # HAM pre-warm: ~6us of dummy matmuls with no input deps keep
                # the PE busy through the initial input DMAs, flipping the
                # clock gate to 8/8 before the real work starts
                wsc = vconst.tile([P, P], dt.bfloat16, tag="wsc")
                nc.vector.memset(wsc, 0.0)
                wps = psp.tile([P, P], dt.float32, tag="ps1", bufs=4)
                for wu in range(56):
                    nc.tensor.matmul(wps, lhsT=wsc, rhs=wsc,
                                     start=(wu == 0), stop=(wu == 55))
                wrd = vconst.tile([P, 1], dt.float32, tag="wrd")
                nc.vector.tensor_scalar_mul(wrd, wps[:, 0:1], 0.0)

                # v MLP1
                hts = []
                for m in range(HID // P):
                    w1t = w1vp.tile([P, 8, P], dt.bfloat16, tag="w1t")
                    nc.sync.dma_start(w1t, w1v_d[m, :, :, :])
                    if m == 0:  # rest of x queued behind the first weight tile
                        nc.sync.dma_start(xt[:, 2:8, :], xT16_d[:, 2:8, :])
                    ps = psp.tile([P, TOK], dt.float32, tag="ps1", bufs=4)
                    for kk in range(8):
                        nc.tensor.matmul(
                            ps, lhsT=w1t[:, kk, :], rhs=xt[:, kk, :],
                            start=(kk == 0), stop=(kk == 7),
                        )
                    bcol = b1_sb[:, 32 + m : 32 + m + 1]
                    sg = kstp.tile([P, TOK], dt.float32, tag="sg")
                    nc.scalar.activation(sg, ps, AF.Sigmoid, bias=bcol)
                    ht = hp.tile([P, TOK], dt.bfloat16, tag="ht")
                    nc.vector.scalar_tensor_tensor(
                        ht, ps, bcol, sg, op0=ALU.add, op1=ALU.mult,
                    )
                    hts.append(ht)

                # second-wave inputs, queued behind the v-MLP1 weight loads
                xt8 = constp.tile([P, 8, TOK], dt.float8e4, tag="xt8")
                nc.sync.dma_start(xt8, xT8_d[:, :, :])
                b1s_sb = constp.tile([P, 96], dt.float32, tag="b1s")
                nc.sync.dma_start(b1s_sb, b1s_d[:, :])
                b2kq_sb = constp.tile([P, 16], dt.float32, tag="b2kq")
                nc.sync.dma_start(b2kq_sb, b2kq_d[:, :])
                b2v_sb = vconst.tile([1, D], dt.bfloat16, tag="b2v")
                nc.sync.dma_start(b2v_sb, b2v_d[:, :])
                ones_sb = vconst.tile([1, P], dt.bfloat16, tag="ones")
                nc.vector.memset(ones_sb, 1.0)

                # v MLP2: token-major v with ones cols; AllGather per tile
                for half in range(2):
                    vps = [
                        psp.tile([P, 2, TOK], dt.float32, tag="ps", name=f"vps{half}{i}")
                        for i in range(2)
                    ]
                    for kk in range(HID // P):
                        w2vt = w2vp.tile([P, D], dt.bfloat16, tag="w2vt")
                        nc.sync.dma_start(w2vt, w2v_d[kk, :, :])
                        for mi in range(2):
                            mt = 2 * half + mi
                            for n2 in range(2):
                                nc.tensor.matmul(
                                    vps[mi][:, n2, :],
                                    lhsT=hts[kk][:, P * mt : P * (mt + 1)],
                                    rhs=w2vt[:, 512 * n2 : 512 * (n2 + 1)],
                                    start=(kk == 0), stop=False,
                                )
                    for mi in range(2):
                        mt = 2 * half + mi
                        for n2 in range(2):
                            nc.tensor.matmul(
                                vps[mi][:, n2, :], lhsT=ones_sb[0:1, 0:P],
                                rhs=b2v_sb[0:1, 512 * n2 : 512 * (n2 + 1)],
                                start=False, stop=True,
                            )
                        vst = vstp.tile([P, VROW], dt.bfloat16, tag="vst")
                        vst3 = vst.rearrange("p (h c) -> p h c", c=DH + 1)
                        for n2 in range(2):
                            src3 = vps[mi][:, n2, :].rearrange("p (h c) -> p h c", c=DH)
                            nc.scalar.activation(
                                vst3[:, 8 * n2 : 8 * n2 + 8, 0:DH], src3, AF.Copy,
                            )
                        nc.vector.memset(vst3[:, :, DH : DH + 1], 1.0)
                        nc.scalar.dma_start(v_dram[P * mt : P * (mt + 1), :], vst)
                        nc.gpsimd.collective_compute(
                            "AllGather", ALU.bypass,
                            replica_groups=groups,
                            ins=[v_dram[P * mt : P * (mt + 1), :].opt()],
                            outs=[vag_dram[4 * P * mt : 4 * P * (mt + 1), :].opt()],
                        )

            # ============ k MLP + q MLP1 (fp8) + attention pools ============
            with (
                tc.tile_pool(name="aconst", bufs=1) as aconst,
                tc.tile_pool(name="kagp", bufs=8) as kagp,
                tc.tile_pool(name="vagp", bufs=16) as vagp,
                tc.tile_pool(name="expp", bufs=8) as expp,
                tc.tile_pool(name="outp", bufs=2) as outp,
            ):
                # k MLP1 (the first k-MLP2 weight loads are queued mid-loop so
                # the DMA queue has them ready at the phase boundary)
                h8 = []
                k_pre = []
                for mp in range(HID // P // 2):
                    mlp1_fp8_unit(w1kq_d["k"], 0, mp, h8)
                    if mp == 10:
                        w2_load(w2kq_d["k"], 0, k_pre)
                    elif mp == 13:
                        w2_load(w2kq_d["k"], 1, k_pre)

                # Load gathered v (whole batch) into SBUF: 16 tiles [128, 1040]
                vag_sb = []
                for vt in range(16):
                    vgt = vagp.tile([P, VROW], dt.bfloat16, tag="vgt")
                    nc.gpsimd.dma_start(vgt, vag_dram[P * vt : P * (vt + 1), :])
                    vag_sb.append(vgt)

                # k MLP2 with a 2-tile w2 prefetch window; the first two q MLP1
                # units are woven in as PE filler for the phase boundary (their
                # hp8/psum needs fit in the spare pool slots - no deadlock)
                h8q = []
                for md in range(D // P):
                    if md + 2 < D // P:
                        w2_load(w2kq_d["k"], md + 2, k_pre)
                    mlp2_fp8_md(0, h8, md, "k", k_pre)
                    mlp1_fp8_unit(w1kq_d["q"], 64, md, h8q)

                # rest of q MLP1
                for mp in range(D // P, HID // P // 2):
                    mlp1_fp8_unit(w1kq_d["q"], 64, mp, h8q)

                masks_sb = aconst.tile([P, NT // 2, 2, P], dt.bfloat16, tag="masks")
                nc.sync.dma_start(masks_sb, masks_d[:, :, :, :])
                neg16 = aconst.tile([1, 64], dt.float16, tag="neg16")
                nc.vector.memset(neg16, -1.0)

                # ---- q MLP2 (fp8) + attention, software-pipelined ----
                q_tiles = []
                q_pre = []
                w2_load(w2kq_d["q"], 0, q_pre)
                w2_load(w2kq_d["q"], 1, q_pre)

                def q_md(md):
                    if md + 2 < D // P:
                        w2_load(w2kq_d["q"], md + 2, q_pre)
                    q_tiles.append(mlp2_fp8_md(8, h8q, md, "q", q_pre))

                def kag_load(pair):
                    kag_sb = []
                    for own in range(4):
                        kgt = kagp.tile([P, TOK], dt.bfloat16, tag="kgt")
                        rt = 8 * (pair // 2) + 2 * own + (pair % 2)
                        nc.gpsimd.dma_start(kgt, kag_dram[P * rt : P * (rt + 1), :])
                        kag_sb.append(kgt)
                    return kag_sb

                def st_unit(pair, kag_sb, j2, exs):
                    """S^T + exp + causal mask for one key-tile pair (both
                    heads).  ex tiles are causal-width [P, 2, TOK-c0]."""
                    blk = (2 * j2) // 4
                    c0 = P * blk  # first valid local query column
                    W = TOK - c0
                    sps_t = [
                        psp.tile([P, 2, TOK], dt.float32, tag="ps", name=f"sps{hh}")
                        for hh in range(2)
                    ]
                    for jj in range(2):
                        j = 2 * j2 + jj
                        own, _ = _owner(j)
                        for hh in range(2):
                            po = 64 * hh
                            nc.tensor.matmul(
                                sps_t[hh][:, jj, c0:TOK],
                                lhsT=kag_sb[own][po : po + 64, P * blk : P * (blk + 1)],
                                rhs=q_tiles[pair][po : po + 64, c0:TOK],
                                start=True, stop=True,
                            )
                    for hh in range(2):
                        ex2 = expp.tile([P, 2, W], dt.bfloat16, tag=f"ex{W}",
                                        bufs=(10 if W == TOK else 9), name=f"ex{hh}")
                        nc.scalar.activation(
                            ex2, sps_t[hh][:, :, c0:TOK], AF.Exp, scale=0.125
                        )
                        # only the diagonal 128-col slot ever needs masking
                        nc.vector.tensor_mul(
                            ex2[:, :, 0:P], ex2[:, :, 0:P], masks_sb[:, j2, :, :],
                        )
                        exs[hh].append(ex2)

                def st_exp_all(pair):
                    """Non-interleaved S^T + exp (prologue only)."""
                    kag_sb = kag_load(pair)
                    exs = [[], []]
                    for j2 in range(NT // 2):
                        st_unit(pair, kag_sb, j2, exs)
                    return exs

                def pv_quarter(pair, exs, hh, jlo):
                    h = 2 * pair + hh
                    pvt = pv_tiles[hh]
                    for j in range(jlo, jlo + NT // 2):
                        own, blk = _owner(j)
                        c0 = P * blk
                        nc.tensor.matmul(
                            pvt[0:65, c0:TOK],
                            lhsT=vag_sb[4 * blk + own][:, 65 * h : 65 * h + 65],
                            rhs=exs[hh][j // 2][:, j % 2, :],
                            start=(j == 0), stop=(j == NT - 1),
                            skip_group_check=True,
                        )

                def pv_div1(pair, pvt, hh):
                    """Softmax division + store for one head.
                    y0 = exp(-ln(d)) on ScalarE (table accuracy), one NR step
                    z = (d*y0 - 2)*y0 = -y1 on GpSimd, broadcast -z across
                    partitions 64..127 of the same PSUM bank via a tiny fp16
                    matmul with a -1 vector, multiply on VectorE, store."""
                    h = 2 * pair + hh
                    den = pvt[64:65, :]
                    ln = outp.tile([1, TOK], dt.float32, tag="ln", bufs=2)
                    nc.scalar.activation(ln, den, AF.Ln)
                    y0 = outp.tile([1, TOK], dt.float32, tag="y0", bufs=2)
                    nc.scalar.activation(y0, ln, AF.Exp, scale=-1.0)
                    t = outp.tile([1, TOK], dt.float32, tag="t", bufs=2)
                    nc.vector.tensor_mul(t, den, y0)
                    z = outp.tile([1, TOK], dt.float16, tag="z", bufs=2)
                    nc.vector.scalar_tensor_tensor(
                        z, t, -2.0, y0, op0=ALU.add, op1=ALU.mult,
                    )
                    nc.tensor.matmul(
                        pvt[64:P, :], lhsT=neg16[0:1, :], rhs=z,
                        start=True, stop=True, skip_group_check=True,
                    )
                    # DVE can read only one PSUM operand; stage the broadcast
                    rbc = outp.tile([64, TOK], dt.float32, tag="rbc", bufs=2)
                    nc.vector.tensor_scalar_mul(rbc, pvt[64:P, :], 1.0)
                    ot = outp.tile([64, TOK], dt.float32, tag="ot", bufs=2)
                    nc.vector.tensor_mul(ot, pvt[0:64, :], rbc)
                    nc.sync.dma_start(o_d[DH * h : DH * (h + 1), :], ot)

                q_md(0)
                exs_cur = st_exp_all(0)
                pend = None
                NP = H // 2
                for pair in range(NP):
                    last = pair + 1 >= NP
                    # At the last iteration no q_md holds a PSUM slot, so the
                    # previous pair's divisions can be deferred to the very
                    # end: the last pair's inline division chains then meet an
                    # empty DVE queue instead of stalling behind them.
                    if pend is not None and not last:
                        for hh in range(2):
                            pv_div1(pend[0], pend[1][hh], hh)
                    if not last:
                        q_md(pair + 1)
                        kag_nx = kag_load(pair + 1)
                    exs_nx = [[], []]
                    pv_tiles = [
                        psp.tile([P, TOK], dt.float32, tag="ps1", bufs=4,
                                 name=f"pvt{hh}")
                        for hh in range(2)
                    ]
                    # weave next pair's S^T units around this pair's PV chunks
                    for step in range(4):
                        if not last:
                            st_unit(pair + 1, kag_nx, 2 * step, exs_nx)
                        pv_quarter(pair, exs_cur, step // 2, (step % 2) * (NT // 2))
                        if not last:
                            st_unit(pair + 1, kag_nx, 2 * step + 1, exs_nx)
                        else:  # drain the division as soon as each head is done
                            if step == 1:
                                pv_div1(pair, pv_tiles[0], 0)
                            elif step == 3:
                                pv_div1(pair, pv_tiles[1], 1)
                    if not last:
                        pend = (pair, pv_tiles)
                        exs_cur = exs_nx
                if pend is not None:
                    for hh in range(2):
                        pv_div1(pend[0], pend[1][hh], hh)
    nc.compile()
    return nc


def _host_inputs(inputs):
    """Build the 8 per-core input maps from the full-problem inputs."""
    x = np.ascontiguousarray(inputs["x"]).astype(np.float32)

    def pack_w1_bf16(w1):
        return np.ascontiguousarray(
            np.asarray(w1).astype(BF16).reshape(8, P, HID // P, P).transpose(2, 1, 0, 3)
        )

    def pack_w1_fp8(w1):
        # [D, HID] -> [HID/(2P), P, 2, 8, P]: HID-tile pairs inside the
        # partition line so one DMA moves 2KB per partition
        w = (np.asarray(w1, np.float32) * WS).astype(F8)
        return np.ascontiguousarray(
            w.reshape(8, P, HID // (2 * P), 2, P).transpose(2, 1, 3, 0, 4)
        )

    def pack_w2_fp8(w2):
        return np.ascontiguousarray(
            (np.asarray(w2, np.float32) * WS).astype(F8)
            .reshape(HID // P, P, D // P, P).transpose(2, 1, 0, 3)
        )

    def b1col(m):
        return np.asarray(inputs[m + "_b1"]).astype(np.float32).reshape(HID // P, P).T

    shared = {
        "w1v": pack_w1_bf16(inputs["v_w1"]),
        "w1k": pack_w1_fp8(inputs["k_w1"]),
        "w1q": pack_w1_fp8(inputs["q_w1"]),
        "w2k": pack_w2_fp8(inputs["k_w2"]),
        "w2q": pack_w2_fp8(inputs["q_w2"]),
        "w2v": np.ascontiguousarray(
            np.asarray(inputs["v_w2"]).astype(BF16).reshape(HID // P, P, D)
        ),
        "b1": np.ascontiguousarray(
            np.concatenate([b1col(m) for m in "kvq"], axis=1)
        ),
        "b1s": np.ascontiguousarray(
            np.concatenate([b1col("k") * WS, b1col("v"), b1col("q") * WS], axis=1)
        ),
        "b2kq": np.ascontiguousarray(
            np.concatenate(
                [np.asarray(inputs[m + "_b2"]).astype(np.float32).reshape(D // P, P).T
                 for m in "kq"], axis=1)
        ),
        "b2v": np.ascontiguousarray(np.asarray(inputs["v_b2"]).astype(BF16).reshape(1, D)),
    }

    in_maps = []
    for c in range(NCORES):
        b, cc = divmod(c, 4)
        tiles = _tiles_for(cc)
        tok = np.concatenate([np.arange(P * t, P * (t + 1)) for t in tiles])
        xT = x[b].T[:, tok]  # [D, TOK]
        xT16 = np.ascontiguousarray(
            xT.astype(BF16).reshape(8, P, TOK).transpose(1, 0, 2)
        )
        xT8 = np.ascontiguousarray(
            xT.astype(F8).reshape(8, P, TOK).transpose(1, 0, 2)
        )
        pk = np.arange(P)
        jj = np.arange(NT)
        fq = np.arange(P)
        # mask for the diagonal slot only: local query col = 128*blk(j) + f
        qglob = np.array(
            [[P * tiles[j // 4] + f for f in fq] for j in jj]
        )  # [NT, P]
        keyglob = P * jj[None, :, None] + pk[:, None, None]  # [P, NT, 1]
        mask = (keyglob <= qglob[None, :, :]).astype(BF16)  # [P, NT, P]
        mask = np.ascontiguousarray(mask.reshape(P, NT // 2, 2, P))
        in_maps.append({**shared, "xT16": xT16, "xT8": xT8, "masks": mask})
    return in_maps


LAST_RESULT = None


def kernel(**inputs):
    global LAST_RESULT
    key = "prog"
    if key not in _CACHE:
        _CACHE[key] = _build_program()
    nc = _CACHE[key]

    from concourse.bass_utils import run_bass_kernel_spmd

    in_maps = _host_inputs(inputs)
    res = run_bass_kernel_spmd(nc, in_maps, core_ids=list(range(NCORES)))
    LAST_RESULT = res

    full = np.zeros((B, S, D), np.float32)
    for c in range(NCORES):
        b, cc = divmod(c, 4)
        o_c = res.results[c]["o"]  # [D, TOK], D-major
        for p, t in enumerate(_tiles_for(cc)):
            full[b, P * t : P * (t + 1), :] = o_c[:, P * p : P * (p + 1)].T
    return full


# revision 59
# speedup vs baseline: 1.0293x; 1.0293x over previous
# Distributed Trainium2 kernel for the QKV-MLP + causal multi-head attention layer.
#
# Problem (all shapes fixed):
#   x [2, 2048, 1024];  q/k/v = MLP(x) with w1 [1024, 4096] (silu) and w2 [4096, 1024]
#   16 heads x 64 dims, causal softmax attention, output [2, 2048, 1024].
#
# Sharding (8 NeuronCores, single SPMD program; per-core behavior differs only
# through per-core input DATA, never through the instruction stream):
#   - Token/data parallel MLPs: each core computes q/k/v for 512 tokens of one
#     batch (cores 0-3 -> batch 0, 4-7 -> batch 1) with full (replicated) weights.
#   - Core quarter cc owns query tiles [cc, 7-cc, 8+cc, 15-cc] (128 tokens each),
#     which balances causal attention cost (sum of key-tiles = 34 for every cc).
#   - k^T (D-major) and v (token-major, with a ones-column per head for the
#     softmax denominator) are AllGathered in 4 chunks each within the 4-core
#     batch group, as soon as each chunk lands in DRAM.
#
# Precision:
#   - q,k MLPs run in fp8 e4m3 with DoubleRow matmuls (2 fp8 MACs/cell/cycle):
#     weights are scaled by 32 on the host (de-scaled via activation scale),
#     activations quantized to e4m3 (32*silu).  Score errors this introduces
#     mostly wash out in the softmax.
#   - v MLP stays bf16: the output inherits v errors directly.
#   - Attention is bf16 with fp32 PSUM accumulation.
#
# Attention engine schedule (software-pipelined by one head-pair):
#   - S^T = k^T q per (head-pair, key-tile-pair) into 2-bank PSUM, exp on
#     ScalarE (ex tiles stored at causal width), diagonal-slot mask on VectorE.
#   - PV is FLIPPED: stationary = v tile [key,65] (with ones column), moving =
#     exp tiles [key, q] -> out^T [65, q] accumulated over 16 key tiles in one
#     PSUM bank.  PV(p) only reads ex tiles masked during iteration p-1, so it
#     never waits on ScalarE; S^T(p+1) units are woven between PV(p) chunks.
#   - softmax division: 1/den via Ln+Exp on ScalarE refined by one
#     Newton-Raphson step on GpSimd, broadcast across partitions with a tiny
#     fp16 matmul into the free partitions 64..127 of the same PSUM bank, one
#     VectorE multiply, store.  No engine sees more than ~1us per head.
#   - Output is written D-major [D, TOK] and transposed on the host.

import numpy as np
import ml_dtypes

B, S, D, HID, H, DH = 2, 2048, 1024, 4096, 16, 64
P = 128
NCORES = 8
TOK = 512           # tokens per core
NT = S // P         # 16 query/key tiles per batch
VROW = H * (DH + 1)  # 1040: v row with a ones column per head
WS = 32.0           # fp8 weight pre-scale (power of two; exact in fp8)

BF16 = ml_dtypes.bfloat16
F8 = ml_dtypes.float8_e4m3  # TRN fp8_exp4-compatible (max normal 240)

_CACHE = {}


def _tiles_for(cc):
    """Global query-tile indices (slot order) owned by quarter cc."""
    return [cc, 7 - cc, 8 + cc, 15 - cc]


def _owner(j):
    """Global tile j -> (owner quarter, slot index)."""
    blk = j // 4
    own = [j, 7 - j, j - 8, 15 - j][blk]
    return own, blk


def _build_program():
    import concourse.bass as bass
    import concourse.mybir as mybir
    import concourse.tile as tile
    from concourse import bacc

    dt = mybir.dt
    AF = mybir.ActivationFunctionType
    DR = mybir.MatmulPerfMode.DoubleRow
    ALU = mybir.AluOpType

    # Bacc (not raw Bass): its compile() splits multi-sem waits into event
    # semaphores, which TRN2 engine instructions require (max 1 wait each).
    nc = bacc.Bacc("TRN2", num_devices=NCORES)

    # ---- I/O ----
    xT16_d = nc.dram_tensor("xT16", [P, 8, TOK], dt.bfloat16, kind="ExternalInput")
    xT8_d = nc.dram_tensor("xT8", [P, 8, TOK], dt.float8e4, kind="ExternalInput")
    w1v_d = nc.dram_tensor("w1v", [HID // P, P, 8, P], dt.bfloat16, kind="ExternalInput")
    # fp8 w1: HID-tile PAIRS with the pair inside the partition line (2KB DMAs)
    w1kq_d = {
        m: nc.dram_tensor(f"w1{m}", [HID // (2 * P), P, 2, 8, P], dt.float8e4,
                          kind="ExternalInput")
        for m in "kq"
    }
    w2kq_d = {
        m: nc.dram_tensor(f"w2{m}", [D // P, P, HID // P, P], dt.float8e4,
                          kind="ExternalInput")
        for m in "kq"
    }
    w2v_d = nc.dram_tensor("w2v", [HID // P, P, D], dt.bfloat16, kind="ExternalInput")
    b1_d = nc.dram_tensor("b1", [P, 96], dt.float32, kind="ExternalInput")
    b1s_d = nc.dram_tensor("b1s", [P, 96], dt.float32, kind="ExternalInput")
    b2kq_d = nc.dram_tensor("b2kq", [P, 16], dt.float32, kind="ExternalInput")
    b2v_d = nc.dram_tensor("b2v", [1, D], dt.bfloat16, kind="ExternalInput")
    masks_d = nc.dram_tensor("masks", [P, NT // 2, 2, P], dt.bfloat16, kind="ExternalInput")
    o_d = nc.dram_tensor("o", [D, TOK], dt.float32, kind="ExternalOutput")

    with tile.TileContext(nc) as tc:
        with (
            tc.tile_pool(name="constp", bufs=1) as constp,
            tc.tile_pool(name="w1p", bufs=8) as w1p,
            tc.tile_pool(name="w2p", bufs=8) as w2p,
            tc.tile_pool(name="hp8", bufs=25) as hp8,
            tc.tile_pool(name="qtp", bufs=2) as qtp,
            tc.tile_pool(name="kstp", bufs=3) as kstp,
            tc.tile_pool(name="psp", bufs=2, space="PSUM") as psp,
            tc.tile_pool(name="dramp", bufs=1, space="DRAM") as dramp,
        ):
            # DRAM bounce buffers for the collectives
            k_dram = dramp.tile([D, TOK], dt.bfloat16, tag="k_dram")
            v_dram = dramp.tile([TOK, VROW], dt.bfloat16, tag="v_dram")
            kag_dram = dramp.tile([4 * D, TOK], dt.bfloat16, tag="kag_dram")
            vag_dram = dramp.tile([4 * TOK, VROW], dt.bfloat16, tag="vag_dram")

            groups = [[0, 1, 2, 3], [4, 5, 6, 7]]

            def mlp1_fp8_unit(w1d, b1col0, mp, hts):
                """One HID-tile pair of the fp8 first linear: appends an
                e4m3(32*silu) tile [P, 2, TOK] to hts (pair dim for DoubleRow)."""
                w1t = w1p.tile([P, 2, 8, P], dt.float8e4, tag="w1t8")
                nc.sync.dma_start(w1t, w1d[mp, :, :, :, :])
                hpair = hp8.tile([P, 2, TOK], dt.float8e4, tag="h8")
                for mi in range(2):
                    m = 2 * mp + mi
                    ps = psp.tile([P, TOK], dt.float32, tag="ps1", bufs=4)
                    for i in range(4):
                        nc.tensor.matmul(
                            ps, lhsT=w1t[:, mi, 2 * i : 2 * i + 2, :],
                            rhs=xt8[:, 2 * i : 2 * i + 2, :],
                            start=(i == 0), stop=(i == 3), perf_mode=DR,
                        )
                    # ps = 32*(x@w1);  sigmoid(ps/32 + b1),  h = (ps+32*b1)*sg
                    bcol = b1_sb[:, b1col0 + m : b1col0 + m + 1]
                    bscol = b1s_sb[:, b1col0 + m : b1col0 + m + 1]
                    sg = kstp.tile([P, TOK], dt.float32, tag="sg")
                    nc.scalar.activation(sg, ps, AF.Sigmoid, bias=bcol, scale=1.0 / WS)
                    nc.vector.scalar_tensor_tensor(
                        hpair[:, mi, :], ps, bscol, sg,
                        op0=ALU.add, op1=ALU.mult,
                    )
                hts.append(hpair)

            def w2_load(w2d, md, pre):
                """Prefetch both 2KB/partition halves of one w2 output tile."""
                for half in range(2):
                    w2t = w2p.tile([P, HID // (2 * P), P], dt.float8e4, tag="w2t8")
                    nc.sync.dma_start(w2t, w2d[md, :, 16 * half : 16 * half + 16, :])
                    pre.append(w2t)

            def mlp2_fp8_md(b2col0, hts, md, dest, pre):
                """One 128-row output tile of the second linear (fp8 DoubleRow):
                ps = (32*silu_h) @ (32*w2); out = Identity(ps/1024 + b2)."""
                ps = psp.tile([P, TOK], dt.float32, tag="ps1", bufs=4)
                for half in range(2):
                    w2t = pre.pop(0)
                    for i in range(8):
                        kp = 8 * half + i
                        nc.tensor.matmul(
                            ps, lhsT=w2t[:, 2 * i : 2 * i + 2, :], rhs=hts[kp],
                            start=(kp == 0), stop=(kp == 15), perf_mode=DR,
                        )
                bcol = b2kq_sb[:, b2col0 + md : b2col0 + md + 1]
                if dest == "k":
                    # bias+descale on DVE (ScalarE is the busier engine here)
                    kst = kstp.tile([P, TOK], dt.bfloat16, tag="kst")
                    nc.vector.tensor_scalar(
                        kst, ps, 1.0 / (WS * WS), bcol,
                        op0=ALU.mult, op1=ALU.add,
                    )
                    # store on the scalar queue: on gpsimd it would queue
                    # behind an AllGather (inter-core wait) and starve the
                    # kst pool -> DVE -> PSUM -> PE chain for ~10us
                    nc.scalar.dma_start(k_dram[P * md : P * (md + 1), :], kst)
                    if md % 2 == 1:
                        # AllGather this 256-row chunk of k^T right away
                        nc.gpsimd.collective_compute(
                            "AllGather", ALU.bypass,
                            replica_groups=groups,
                            ins=[k_dram[P * (md - 1) : P * (md + 1), :].opt()],
                            outs=[kag_dram[4 * P * (md - 1) : 4 * P * (md + 1), :].opt()],
                        )
                    return None
                qt = qtp.tile([P, TOK], dt.bfloat16, tag="qt")
                nc.vector.tensor_scalar(
                    qt, ps, 1.0 / (WS * WS), bcol, op0=ALU.mult, op1=ALU.add,
                )
                return qt

            # ================= v MLP (bf16), scoped pools =================
            with (
                tc.tile_pool(name="vconst", bufs=1) as vconst,
                tc.tile_pool(name="w1vp", bufs=2) as w1vp,
                tc.tile_pool(name="w2vp", bufs=6) as w2vp,
                tc.tile_pool(name="hp", bufs=32) as hp,
                tc.tile_pool(name="vstp", bufs=2) as vstp,
            ):
                xt = vconst.tile([P, 8, TOK], dt.bfloat16, tag="xt")
                nc.sync.dma_start(xt[:, 0:2, :], xT16_d[:, 0:2, :])
                b1_sb = constp.tile([P, 96], dt.float32, tag="b1")
                nc.sync.dma_start(b1_sb, b1_d[:, :])

                # HAM pre-warm: ~6us of dummy matmuls with no input deps keep
                # the PE busy through the initial input DMAs, flipping the
                # clock gate to 8/8 before the real work starts
                wsc = vconst.tile([P, P], dt.bfloat16, tag="wsc")
                nc.vector.memset(wsc, 0.0)
                wps = psp.tile([P, P], dt.float32, tag="ps1", bufs=4)
                for wu in range(56):
                    nc.tensor.matmul(wps, lhsT=wsc, rhs=wsc,
                                     start=(wu == 0), stop=(wu == 55))
                wrd = vconst.tile([P, 1], dt.float32, tag="wrd")
                nc.vector.tensor_scalar_mul(wrd, wps[:, 0:1], 0.0)

                # v MLP1
                hts = []
                for m in range(HID // P):
                    w1t = w1vp.tile([P, 8, P], dt.bfloat16, tag="w1t")
                    nc.sync.dma_start(w1t, w1v_d[m, :, :, :])
                    if m == 0:  # rest of x queued behind the first weight tile
                        nc.sync.dma_start(xt[:, 2:8, :], xT16_d[:, 2:8, :])
                    ps = psp.tile([P, TOK], dt.float32, tag="ps1", bufs=4)
                    for kk in range(8):
                        nc.tensor.matmul(
                            ps, lhsT=w1t[:, kk, :], rhs=xt[:, kk, :],
                            start=(kk == 0), stop=(kk == 7),
                        )
                    bcol = b1_sb[:, 32 + m : 32 + m + 1]
                    sg = kstp.tile([P, TOK], dt.float32, tag="sg")
                    nc.scalar.activation(sg, ps, AF.Sigmoid, bias=bcol)
                    ht = hp.tile([P, TOK], dt.bfloat16, tag="ht")
                    nc.vector.scalar_tensor_tensor(
                        ht, ps, bcol, sg, op0=ALU.add, op1=ALU.mult,
                    )
                    hts.append(ht)

                # second-wave inputs, queued behind the v-MLP1 weight loads
                xt8 = constp.tile([P, 8, TOK], dt.float8e4, tag="xt8")
                nc.sync.dma_start(xt8, xT8_d[:, :, :])
                b1s_sb = constp.tile([P, 96], dt.float32, tag="b1s")
                nc.sync.dma_start(b1s_sb, b1s_d[:, :])
                b2kq_sb = constp.tile([P, 16], dt.float32, tag="b2kq")
                nc.sync.dma_start(b2kq_sb, b2kq_d[:, :])
                b2v_sb = vconst.tile([1, D], dt.bfloat16, tag="b2v")
                nc.sync.dma_start(b2v_sb, b2v_d[:, :])
                ones_sb = vconst.tile([1, P], dt.bfloat16, tag="ones")
                nc.vector.memset(ones_sb, 1.0)

                # v MLP2: token-major v with ones cols; AllGather per tile
                for half in range(2):
                    vps = [
                        psp.tile([P, 2, TOK], dt.float32, tag="ps", name=f"vps{half}{i}")
                        for i in range(2)
                    ]
                    for kk in range(HID // P):
                        w2vt = w2vp.tile([P, D], dt.bfloat16, tag="w2vt")
                        nc.sync.dma_start(w2vt, w2v_d[kk, :, :])
                        for mi in range(2):
                            mt = 2 * half + mi
                            for n2 in range(2):
                                nc.tensor.matmul(
                                    vps[mi][:, n2, :],
                                    lhsT=hts[kk][:, P * mt : P * (mt + 1)],
                                    rhs=w2vt[:, 512 * n2 : 512 * (n2 + 1)],
                                    start=(kk == 0), stop=False,
                                )
                    for mi in range(2):
                        mt = 2 * half + mi
                        for n2 in range(2):
                            nc.tensor.matmul(
                                vps[mi][:, n2, :], lhsT=ones_sb[0:1, 0:P],
                                rhs=b2v_sb[0:1, 512 * n2 : 512 * (n2 + 1)],
                                start=False, stop=True,
                            )
                        vst = vstp.tile([P, VROW], dt.bfloat16, tag="vst")
                        vst3 = vst.rearrange("p (h c) -> p h c", c=DH + 1)
                        for n2 in range(2):
                            src3 = vps[mi][:, n2, :].rearrange("p (h c) -> p h c", c=DH)
                            nc.scalar.activation(
                                vst3[:, 8 * n2 : 8 * n2 + 8, 0:DH], src3, AF.Copy,
                            )
                        nc.vector.memset(vst3[:, :, DH : DH + 1], 1.0)
                        nc.scalar.dma_start(v_dram[P * mt : P * (mt + 1), :], vst)
                        nc.gpsimd.collective_compute(
                            "AllGather", ALU.bypass,
                            replica_groups=groups,
                            ins=[v_dram[P * mt : P * (mt + 1), :].opt()],
                            outs=[vag_dram[4 * P * mt : 4 * P * (mt + 1), :].opt()],
                        )

            # ============ k MLP + q MLP1 (fp8) + attention pools ============
            with (
                tc.tile_pool(name="aconst", bufs=1) as aconst,
                tc.tile_pool(name="kagp", bufs=8) as kagp,
                tc.tile_pool(name="vagp", bufs=16) as vagp,
                tc.tile_pool(name="expp", bufs=8) as expp,
                tc.tile_pool(name="outp", bufs=2) as outp,
            ):
                # k MLP1 (the first k-MLP2 weight loads are queued mid-loop so
                # the DMA queue has them ready at the phase boundary)
                h8 = []
                k_pre = []
                for mp in range(HID // P // 2):
                    mlp1_fp8_unit(w1kq_d["k"], 0, mp, h8)
                    if mp == 10:
                        w2_load(w2kq_d["k"], 0, k_pre)
                    elif mp == 13:
                        w2_load(w2kq_d["k"], 1, k_pre)

                # Load gathered v (whole batch) into SBUF: 16 tiles [128, 1040]
                vag_sb = []
                for vt in range(16):
                    vgt = vagp.tile([P, VROW], dt.bfloat16, tag="vgt")
                    nc.gpsimd.dma_start(vgt, vag_dram[P * vt : P * (vt + 1), :])
                    vag_sb.append(vgt)

                # k MLP2 with a 2-tile w2 prefetch window; the first two q MLP1
                # units are woven in as PE filler for the phase boundary (their
                # hp8/psum needs fit in the spare pool slots - no deadlock)
                h8q = []
                for md in range(D // P):
                    if md + 2 < D // P:
                        w2_load(w2kq_d["k"], md + 2, k_pre)
                    mlp2_fp8_md(0, h8, md, "k", k_pre)
                    mlp1_fp8_unit(w1kq_d["q"], 64, md, h8q)

                # rest of q MLP1
                for mp in range(D // P, HID // P // 2):
                    mlp1_fp8_unit(w1kq_d["q"], 64, mp, h8q)

                masks_sb = aconst.tile([P, NT // 2, 2, P], dt.bfloat16, tag="masks")
                nc.sync.dma_start(masks_sb, masks_d[:, :, :, :])
                neg16 = aconst.tile([1, 64], dt.float16, tag="neg16")
                nc.vector.memset(neg16, -1.0)

                # ---- q MLP2 (fp8) + attention, software-pipelined ----
                q_tiles = []
                q_pre = []
                w2_load(w2kq_d["q"], 0, q_pre)
                w2_load(w2kq_d["q"], 1, q_pre)

                def q_md(md):
                    if md + 2 < D // P:
                        w2_load(w2kq_d["q"], md + 2, q_pre)
                    q_tiles.append(mlp2_fp8_md(8, h8q, md, "q", q_pre))

                def kag_load(pair):
                    kag_sb = []
                    for own in range(4):
                        kgt = kagp.tile([P, TOK], dt.bfloat16, tag="kgt")
                        rt = 8 * (pair // 2) + 2 * own + (pair % 2)
                        nc.gpsimd.dma_start(kgt, kag_dram[P * rt : P * (rt + 1), :])
                        kag_sb.append(kgt)
                    return kag_sb

                def st_unit(pair, kag_sb, j2, exs):
                    """S^T + exp + causal mask for one key-tile pair (both
                    heads).  ex tiles are causal-width [P, 2, TOK-c0]."""
                    blk = (2 * j2) // 4
                    c0 = P * blk  # first valid local query column
                    W = TOK - c0
                    sps_t = [
                        psp.tile([P, 2, TOK], dt.float32, tag="ps", name=f"sps{hh}")
                        for hh in range(2)
                    ]
                    for jj in range(2):
                        j = 2 * j2 + jj
                        own, _ = _owner(j)
                        for hh in range(2):
                            po = 64 * hh
                            nc.tensor.matmul(
                                sps_t[hh][:, jj, c0:TOK],
                                lhsT=kag_sb[own][po : po + 64, P * blk : P * (blk + 1)],
                                rhs=q_tiles[pair][po : po + 64, c0:TOK],
                                start=True, stop=True,
                            )
                    for hh in range(2):
                        ex2 = expp.tile([P, 2, W], dt.bfloat16, tag=f"ex{W}",
                                        bufs=(10 if W == TOK else 9), name=f"ex{hh}")
                        nc.scalar.activation(
                            ex2, sps_t[hh][:, :, c0:TOK], AF.Exp, scale=0.125
                        )
                        # only the diagonal 128-col slot ever needs masking
                        nc.vector.tensor_mul(
                            ex2[:, :, 0:P], ex2[:, :, 0:P], masks_sb[:, j2, :, :],
                        )
                        exs[hh].append(ex2)

                def st_exp_all(pair):
                    """Non-interleaved S^T + exp (prologue only)."""
                    kag_sb = kag_load(pair)
                    exs = [[], []]
                    for j2 in range(NT // 2):
                        st_unit(pair, kag_sb, j2, exs)
                    return exs

                def pv_quarter(pair, exs, hh, jlo):
                    h = 2 * pair + hh
                    pvt = pv_tiles[hh]
                    for j in range(jlo, jlo + NT // 2):
                        own, blk = _owner(j)
                        c0 = P * blk
                        nc.tensor.matmul(
                            pvt[0:65, c0:TOK],
                            lhsT=vag_sb[4 * blk + own][:, 65 * h : 65 * h + 65],
                            rhs=exs[hh][j // 2][:, j % 2, :],
                            start=(j == 0), stop=(j == NT - 1),
                            skip_group_check=True,
                        )

                def pv_div1(pair, pvt, hh):
                    """Softmax division + store for one head.
                    y0 = exp(-ln(d)) on ScalarE (table accuracy), one NR step
                    z = (d*y0 - 2)*y0 = -y1 on GpSimd, broadcast -z across
                    partitions 64..127 of the same PSUM bank via a tiny fp16
                    matmul with a -1 vector, multiply on VectorE, store."""
                    h = 2 * pair + hh
                    den = pvt[64:65, :]
                    ln = outp.tile([1, TOK], dt.float32, tag="ln", bufs=2)
                    nc.scalar.activation(ln, den, AF.Ln)
                    y0 = outp.tile([1, TOK], dt.float32, tag="y0", bufs=2)
                    nc.scalar.activation(y0, ln, AF.Exp, scale=-1.0)
                    t = outp.tile([1, TOK], dt.float32, tag="t", bufs=2)
                    nc.vector.tensor_mul(t, den, y0)
                    z = outp.tile([1, TOK], dt.float16, tag="z", bufs=2)
                    nc.vector.scalar_tensor_tensor(
                        z, t, -2.0, y0, op0=ALU.add, op1=ALU.mult,
                    )
                    nc.tensor.matmul(
                        pvt[64:P, :], lhsT=neg16[0:1, :], rhs=z,
                        start=True, stop=True, skip_group_check=True,
                    )
                    # DVE can read only one PSUM operand; stage the broadcast
                    rbc = outp.tile([64, TOK], dt.float32, tag="rbc", bufs=2)
                    nc.vector.tensor_scalar_mul(rbc, pvt[64:P, :], 1.0)
                    ot = outp.tile([64, TOK], dt.float32, tag="ot", bufs=2)
                    nc.vector.tensor_mul(ot, pvt[0:64, :], rbc)
                    nc.sync.dma_start(o_d[DH * h : DH * (h + 1), :], ot)

                q_md(0)
                exs_cur = st_exp_all(0)
                pend = None
                NP = H // 2
                for pair in range(NP):
                    last = pair + 1 >= NP
                    # At the last iteration no q_md holds a PSUM slot, so the
                    # previous pair's divisions can be deferred to the very
                    # end: the last pair's inline division chains then meet an
                    # empty DVE queue instead of stalling behind them.
                    if pend is not None and not last:
                        for hh in range(2):
                            pv_div1(pend[0], pend[1][hh], hh)
                    if not last:
                        q_md(pair + 1)
                        kag_nx = kag_load(pair + 1)
                    exs_nx = [[], []]
                    pv_tiles = [
                        psp.tile([P, TOK], dt.float32, tag="ps1", bufs=4,
                                 name=f"pvt{hh}")
                        for hh in range(2)
                    ]
                    # weave next pair's S^T units around this pair's PV chunks
                    for step in range(4):
                        if not last:
                            st_unit(pair + 1, kag_nx, 2 * step, exs_nx)
                        pv_quarter(pair, exs_cur, step // 2, (step % 2) * (NT // 2))
                        if not last:
                            st_unit(pair + 1, kag_nx, 2 * step + 1, exs_nx)
                    if not last:
                        pend = (pair, pv_tiles)
                        exs_cur = exs_nx
                    else:
                        # last pair: emit divisions only after ALL PV matmuls -
                        # an earlier broadcast MM would block PV h1 in the PE's
                        # in-order queue while the ln/exp/NR chain drains
                        for hh in range(2):
                            pv_div1(pair, pv_tiles[hh], hh)
                if pend is not None:
                    for hh in range(2):
                        pv_div1(pend[0], pend[1][hh], hh)
    nc.compile()
    return nc


def _host_inputs(inputs):
    """Build the 8 per-core input maps from the full-problem inputs."""
    x = np.ascontiguousarray(inputs["x"]).astype(np.float32)

    def pack_w1_bf16(w1):
        return np.ascontiguousarray(
            np.asarray(w1).astype(BF16).reshape(8, P, HID // P, P).transpose(2, 1, 0, 3)
        )

    def pack_w1_fp8(w1):
        # [D, HID] -> [HID/(2P), P, 2, 8, P]: HID-tile pairs inside the
        # partition line so one DMA moves 2KB per partition
        w = (np.asarray(w1, np.float32) * WS).astype(F8)
        return np.ascontiguousarray(
            w.reshape(8, P, HID // (2 * P), 2, P).transpose(2, 1, 3, 0, 4)
        )

    def pack_w2_fp8(w2):
        return np.ascontiguousarray(
            (np.asarray(w2, np.float32) * WS).astype(F8)
            .reshape(HID // P, P, D // P, P).transpose(2, 1, 0, 3)
        )

    def b1col(m):
        return np.asarray(inputs[m + "_b1"]).astype(np.float32).reshape(HID // P, P).T

    shared = {
        "w1v": pack_w1_bf16(inputs["v_w1"]),
        "w1k": pack_w1_fp8(inputs["k_w1"]),
        "w1q": pack_w1_fp8(inputs["q_w1"]),
        "w2k": pack_w2_fp8(inputs["k_w2"]),
        "w2q": pack_w2_fp8(inputs["q_w2"]),
        "w2v": np.ascontiguousarray(
            np.asarray(inputs["v_w2"]).astype(BF16).reshape(HID // P, P, D)
        ),
        "b1": np.ascontiguousarray(
            np.concatenate([b1col(m) for m in "kvq"], axis=1)
        ),
        "b1s": np.ascontiguousarray(
            np.concatenate([b1col("k") * WS, b1col("v"), b1col("q") * WS], axis=1)
        ),
        "b2kq": np.ascontiguousarray(
            np.concatenate(
                [np.asarray(inputs[m + "_b2"]).astype(np.float32).reshape(D // P, P).T
                 for m in "kq"], axis=1)
        ),
        "b2v": np.ascontiguousarray(np.asarray(inputs["v_b2"]).astype(BF16).reshape(1, D)),
    }

    in_maps = []
    for c in range(NCORES):
        b, cc = divmod(c, 4)
        tiles = _tiles_for(cc)
        tok = np.concatenate([np.arange(P * t, P * (t + 1)) for t in tiles])
        xT = x[b].T[:, tok]  # [D, TOK]
        xT16 = np.ascontiguousarray(
            xT.astype(BF16).reshape(8, P, TOK).transpose(1, 0, 2)
        )
        xT8 = np.ascontiguousarray(
            xT.astype(F8).reshape(8, P, TOK).transpose(1, 0, 2)
        )
        pk = np.arange(P)
        jj = np.arange(NT)
        fq = np.arange(P)
        # mask for the diagonal slot only: local query col = 128*blk(j) + f
        qglob = np.array(
            [[P * tiles[j // 4] + f for f in fq] for j in jj]
        )  # [NT, P]
        keyglob = P * jj[None, :, None] + pk[:, None, None]  # [P, NT, 1]
        mask = (keyglob <= qglob[None, :, :]).astype(BF16)  # [P, NT, P]
        mask = np.ascontiguousarray(mask.reshape(P, NT // 2, 2, P))
        in_maps.append({**shared, "xT16": xT16, "xT8": xT8, "masks": mask})
    return in_maps


LAST_RESULT = None


def kernel(**inputs):
    global LAST_RESULT
    key = "prog"
    if key not in _CACHE:
        _CACHE[key] = _build_program()
    nc = _CACHE[key]

    from concourse.bass_utils import run_bass_kernel_spmd

    in_maps = _host_inputs(inputs)
    res = run_bass_kernel_spmd(nc, in_maps, core_ids=list(range(NCORES)))
    LAST_RESULT = res

    full = np.zeros((B, S, D), np.float32)
    for c in range(NCORES):
        b, cc = divmod(c, 4)
        o_c = res.results[c]["o"]  # [D, TOK], D-major
        for p, t in enumerate(_tiles_for(cc)):
            full[b, P * t : P * (t + 1), :] = o_c[:, P * p : P * (p + 1)].T
    return full
